# revision 9
# baseline (speedup 1.0000x reference)
"""DotPredictor on 8 TRN2 cores — src-run dedup over dma_gather.

score[e] = <h[src[e]], h[dst[e]]>.

Measured fact: the kernel is 100% gather-bound (DVE fully hidden; a
gather-only variant times identically to the full kernel). Cost per
dma_gather instruction ~= 994 ns fixed + ~1.2 ns/idx of Q7 descriptor
generation, serialized on the Pool engine; num_idxs > 1024 hard-faults the
ucode (verified at 2048 and 8192), so fewer descriptors is the only lever.

Sharding: core c owns src nodes [12500c, 12500(c+1)) and the edges whose src
falls there (counts vary ±~300; static caps are maxed over cores, padded
slots discarded by the host). Per core, edges are sorted by (dst_bucket,
src); equal-src runs within a (core, dst_bucket) segment share ONE gathered
src row (~39k runs vs 75k edges). dst rows are gathered per edge (~81k
slots incl. padding). Total ~120k descriptors/core vs 150k for the
all-pairs version.

Layout: runs are length-sorted (desc) and packed 128 to a group; group g
owns l_g = max run length columns of the slot grid. Edge j of the run at
partition p of group g sits at slot (p, group_col0 + j). dst gathers fill
slots column-major; the src row of each run is gathered once into a
resident [128, G, D] tile (indices are hrange-local, so int16-safe with no
src bucketing and tail-free 1024-chunking). DVE multiplies each dst slot
column block by its group's broadcast src column and row-reduces into
scores; the host inverse-maps slots to edges.

h is bf16 (halves SBUF residency; descriptor cost is unchanged).
"""

import ml_dtypes
import numpy as np

import concourse.bacc as bacc
import concourse.mybir as mybir
import concourse.tile as tile

N_CORES = 8
N_NODES = 100000
N_EDGES = 600000
D = 128
P = 128

NODES_PER_CORE = N_NODES // N_CORES   # 12500
RANGE_CAP = 12544                     # hrange rows (>= NODES_PER_CORE)
N_BUCKETS = 4
BUCKET = 25000                        # dst bucket size (int16-safe)
K_GATHER = 1024                       # SWDGE ring-safe chunk
N_QUEUES = 4
DST_BUFS = 2                          # rotating dst gather tiles


def plan(all_src, all_dst):
    """Host layout. Returns (static_plan, per_core_data)."""
    core_of = all_src // NODES_PER_CORE
    per_core = []
    runs_len = [[None] * N_BUCKETS for _ in range(N_CORES)]
    for c in range(N_CORES):
        eidx = np.nonzero(core_of == c)[0]
        src = all_src[eidx]
        dst = all_dst[eidx]
        db = dst // BUCKET
        order = np.lexsort((src, db))          # sort by (dst_bucket, src)
        src, dst, db, eidx = src[order], dst[order], db[order], eidx[order]
        segs = []
        for b in range(N_BUCKETS):
            m = db == b
            sb, dbv, eb = src[m], dst[m], eidx[m]
            # runs of equal src (already sorted by src within bucket)
            uniq, start, cnt = np.unique(sb, return_index=True,
                                         return_counts=True)
            # order runs by length desc (stable)
            ro = np.argsort(-cnt, kind="stable")
            segs.append((uniq[ro], start[ro], cnt[ro], sb, dbv, eb))
            runs_len[c][b] = cnt[ro]
        per_core.append(segs)

    # static caps: per bucket, group count and per-group column capacity
    ell_hat = []
    g_cap = []
    for b in range(N_BUCKETS):
        gmax = max(-(-len(runs_len[c][b]) // P) for c in range(N_CORES))
        g_cap.append(gmax)
        ell = np.zeros(gmax, dtype=np.int64)
        for c in range(N_CORES):
            cnt = runs_len[c][b]
            for g in range(-(-len(cnt) // P)):
                ell[g] = max(ell[g], cnt[g * P])   # desc-sorted: first is max
        ell_hat.append(ell)
    # dst columns per bucket (tails chunk at 128-idx granularity)
    dst_cols = [int(e.sum()) for e in ell_hat]
    src_cols = [int(g) for g in g_cap]            # one src col per group
    return (ell_hat, g_cap, dst_cols, src_cols), per_core


def _chunks(total_idx):
    out = [K_GATHER] * (total_idx // K_GATHER)
    t = total_idx % K_GATHER
    if t:
        out.append(t)                  # already multiple of 128
    return out


def build_program(static_plan, repeats=1, do_dve=True, do_gather=True):
    ell_hat, g_cap, dst_cols, src_cols = static_plan
    G_tot = sum(src_cols)
    C_tot = sum(dst_cols)
    sidx_cols = G_tot * P // 16        # wrapped idx cols
    didx_cols = C_tot * P // 16

    nc = bacc.Bacc("TRN2", target_bir_lowering=False, debug=False,
                   num_swdge_queues=N_QUEUES)
    h = nc.dram_tensor("h", [N_NODES, D], mybir.dt.bfloat16,
                       kind="ExternalInput")
    hrange = nc.dram_tensor("hrange", [RANGE_CAP, D], mybir.dt.bfloat16,
                            kind="ExternalInput")
    sidx = nc.dram_tensor("sidx", [P, sidx_cols], mybir.dt.int16,
                          kind="ExternalInput")
    didx = nc.dram_tensor("didx", [P, didx_cols], mybir.dt.int16,
                          kind="ExternalInput")
    out = nc.dram_tensor("scores", [P, C_tot], mybir.dt.float32,
                         kind="ExternalOutput")

    qn = [0]

    def q():
        qn[0] += 1
        return qn[0] % N_QUEUES

    with tile.TileContext(nc) as tc:
        with (
            tc.tile_pool(name="idxp", bufs=1) as idx_pool,
            tc.tile_pool(name="sr", bufs=1) as src_pool,
            tc.tile_pool(name="sc", bufs=1) as sc_pool,
            tc.tile_pool(name="dp", bufs=DST_BUFS) as dpool,
        ):
            sidx_t = idx_pool.tile([P, sidx_cols], mybir.dt.int16)
            didx_t = idx_pool.tile([P, didx_cols], mybir.dt.int16)
            nc.sync.dma_start(out=sidx_t[:], in_=sidx[:])
            nc.sync.dma_start(out=didx_t[:], in_=didx[:])
            scores = sc_pool.tile([P, C_tot], mybir.dt.float32)
            nc.vector.memset(scores[:], 0.0)
            src_res = src_pool.tile([P, G_tot, D], mybir.dt.bfloat16)

            max_bucket_cols = max(dst_cols)
            for _rep in range(repeats):
                # 1) src rows: one flat gather stream from hrange
                i0, col = 0, 0
                for kk in _chunks(G_tot * P):
                    ct = kk // P
                    S = kk // 16
                    if do_gather:
                        nc.gpsimd.dma_gather(
                            out_ap=src_res[:, col:col + ct, :],
                            in_ap=hrange[:, :],
                            idxs_ap=sidx_t[:, i0:i0 + S],
                            num_idxs=kk, num_idxs_reg=kk,
                            elem_size=D, queue_num=q())
                    i0 += S
                    col += ct
                if not do_gather:
                    nc.gpsimd.memset(src_res[:], 0.0)

                # 2) per dst bucket: gather dst slots, then DVE
                i0 = 0
                bcol0 = 0          # scores column base of bucket
                gcol0 = 0          # src_res column base of bucket
                for b in range(N_BUCKETS):
                    h_b = h[b * BUCKET:(b + 1) * BUCKET, :]
                    cb = dst_cols[b]
                    dtile = dpool.tile([P, max_bucket_cols, D],
                                       mybir.dt.bfloat16, tag="d")
                    col = 0
                    for kk in _chunks(cb * P):
                        ct = kk // P
                        S = kk // 16
                        if do_gather:
                            nc.gpsimd.dma_gather(
                                out_ap=dtile[:, col:col + ct, :],
                                in_ap=h_b,
                                idxs_ap=didx_t[:, i0:i0 + S],
                                num_idxs=kk, num_idxs_reg=kk,
                                elem_size=D, queue_num=q())
                        else:
                            nc.gpsimd.memset(dtile[:, col:col + ct, :], 0.0)
                        i0 += S
                        col += ct
                    if do_dve:
                        # group spans: merge consecutive groups with l==1
                        ell = ell_hat[b]
                        g = 0
                        dcol = 0
                        while g < len(ell):
                            l = int(ell[g])
                            if l == 1:
                                m = len(ell) - g     # desc-sorted: rest are 1
                                d_sl = dtile[:, dcol:dcol + m, :]
                                s_sl = src_res[:, gcol0 + g:gcol0 + g + m, :]
                                nc.vector.tensor_mul(out=d_sl, in0=d_sl,
                                                     in1=s_sl)
                                nc.vector.tensor_reduce(
                                    out=scores[:, bcol0 + dcol:
                                               bcol0 + dcol + m],
                                    in_=d_sl, axis=mybir.AxisListType.X,
                                    op=mybir.AluOpType.add)
                                dcol += m
                                g += m
                            else:
                                d_sl = dtile[:, dcol:dcol + l, :]
                                s_sl = src_res[:, gcol0 + g:gcol0 + g + 1, :]
                                nc.vector.tensor_mul(
                                    out=d_sl, in0=d_sl,
                                    in1=s_sl.to_broadcast([P, l, D]))
                                nc.vector.tensor_reduce(
                                    out=scores[:, bcol0 + dcol:
                                               bcol0 + dcol + l],
                                    in_=d_sl, axis=mybir.AxisListType.X,
                                    op=mybir.AluOpType.add)
                                dcol += l
                                g += 1
                    bcol0 += cb
                    gcol0 += g_cap[b]

            nc.sync.dma_start(out=out[:], in_=scores[:])
    nc.compile()
    return nc


def _wrap_block(flat_i16):
    k = flat_i16.shape[0]
    w = flat_i16.reshape(k // 16, 16).T
    return np.ascontiguousarray(np.tile(w, (8, 1)))


def _wrap_stream(flat_i16):
    segs = []
    off = 0
    for kk in _chunks(flat_i16.shape[0]):
        segs.append(_wrap_block(flat_i16[off:off + kk]))
        off += kk
    return np.concatenate(segs, axis=1)


def make_core_inputs(core_id, segs, static_plan, h):
    ell_hat, g_cap, dst_cols, src_cols = static_plan
    h16 = np.asarray(h, dtype=ml_dtypes.bfloat16)
    lo = core_id * NODES_PER_CORE
    hr = np.zeros((RANGE_CAP, D), dtype=ml_dtypes.bfloat16)
    hi = min(lo + RANGE_CAP, N_NODES)
    hr[:hi - lo] = h16[lo:hi]

    G_tot = sum(g_cap)
    C_tot = sum(dst_cols)
    src_flat = (np.arange(G_tot * P, dtype=np.int64) * 97) % NODES_PER_CORE
    dst_flat = np.empty(C_tot * P, dtype=np.int64)

    e_glob = []          # original edge index per slot-entry
    e_slot_p = []
    e_slot_col = []

    gcol0 = 0
    bcol0 = 0
    for b in range(N_BUCKETS):
        uniq, start, cnt, sb, dbv, eb = segs[b]
        ell = ell_hat[b]
        nr = len(uniq)
        # src slots: run r -> (partition r%P at flat pos g*P + p)
        r = np.arange(nr)
        src_flat[gcol0 * P + r] = uniq - lo
        # dst slots, column-major within bucket
        seg = (np.arange(dst_cols[b] * P, dtype=np.int64) * 89) % BUCKET
        col_of_g = np.concatenate([[0], np.cumsum(ell)])
        for g in range(-(-nr // P)):
            rr = np.arange(g * P, min((g + 1) * P, nr))
            for j in range(int(ell[g])):
                sel = rr[cnt[rr] > j]
                if len(sel) == 0:
                    continue
                p = sel % P
                col = col_of_g[g] + j
                eix = start[sel] + j
                seg[col * P + p] = dbv[eix] - b * BUCKET
                e_glob.append(eb[eix])
                e_slot_p.append(p)
                e_slot_col.append(np.full(len(p), bcol0 + col))
        dst_flat[bcol0 * P:(bcol0 + dst_cols[b]) * P] = seg
        gcol0 += g_cap[b]
        bcol0 += dst_cols[b]

    sidx = _wrap_stream(src_flat.astype(np.int16))
    didx = _wrap_stream(dst_flat.astype(np.int16))
    mapping = (np.concatenate(e_glob), np.concatenate(e_slot_p),
               np.concatenate(e_slot_col))
    return ({"h": h16, "hrange": hr, "sidx": np.ascontiguousarray(sidx),
             "didx": np.ascontiguousarray(didx)}, mapping)


def run(edge_index, h, pad_value=0):
    from concourse.bass_utils import run_bass_kernel_spmd

    h = np.ascontiguousarray(np.asarray(h), dtype=np.float32)
    all_src = np.asarray(edge_index[0], dtype=np.int64)
    all_dst = np.asarray(edge_index[1], dtype=np.int64)
    static_plan, per_core = plan(all_src, all_dst)
    nc = build_program(static_plan)

    in_maps, mappings = [], []
    for c in range(N_CORES):
        m, mapping = make_core_inputs(c, per_core[c], static_plan, h)
        in_maps.append(m)
        mappings.append(mapping)

    res = run_bass_kernel_spmd(nc, in_maps, core_ids=list(range(N_CORES)))

    out = np.empty(N_EDGES, dtype=np.float32)
    for c in range(N_CORES):
        eg, pp, cc = mappings[c]
        scores = res.results[c]["scores"]
        out[eg] = scores[pp, cc]
    return out, res


def kernel(edge_index, h):
    out, _ = run(edge_index, h)
    return out


# revision 10
# speedup vs baseline: 1.4345x; 1.4345x over previous
"""DotPredictor on 8 TRN2 cores — src-run dedup over dma_gather.

score[e] = <h[src[e]], h[dst[e]]>.

Measured fact: the kernel is 100% gather-bound (DVE fully hidden; a
gather-only variant times identically to the full kernel). Cost per
dma_gather instruction ~= 994 ns fixed + ~1.2 ns/idx of Q7 descriptor
generation, serialized on the Pool engine; num_idxs > 1024 hard-faults the
ucode (verified at 2048 and 8192), so fewer descriptors is the only lever.

Sharding: core c owns src nodes [12500c, 12500(c+1)) and the edges whose src
falls there (counts vary ±~300; static caps are maxed over cores, padded
slots discarded by the host). Per core, edges are sorted by (dst_bucket,
src) with dst buckets of [32768, 32768, 32768, 1696] rows (3 full int16
windows densify the src runs); equal-src runs within a (core, dst_bucket)
segment share ONE gathered src row (~34k runs vs 75k edges). dst rows are
gathered per edge (~79k slots incl. padding). Total ~113k descriptors/core
vs 150k for the all-pairs version; measured ~235 us vs ~340 us.

Layout: runs are length-sorted (desc) and packed 128 to a group; group g
owns l_g = max run length columns of the slot grid. Edge j of the run at
partition p of group g sits at slot (p, group_col0 + j). dst gathers fill
slots column-major; the src row of each run is gathered once into a
resident [128, G, D] tile (indices are hrange-local, so int16-safe with no
src bucketing and tail-free 1024-chunking). DVE multiplies each dst slot
column block by its group's broadcast src column and row-reduces into
scores; the host inverse-maps slots to edges.

h is bf16 (halves SBUF residency; descriptor cost is unchanged).
"""

import ml_dtypes
import numpy as np

import concourse.bacc as bacc
import concourse.mybir as mybir
import concourse.tile as tile

N_CORES = 8
N_NODES = 100000
N_EDGES = 600000
D = 128
P = 128

NODES_PER_CORE = N_NODES // N_CORES   # 12500
RANGE_CAP = 12544                     # hrange rows (>= NODES_PER_CORE)
N_BUCKETS = 4
B_EDGES = [0, 32768, 65536, 98304, 100000]   # dst bucket bounds (int16-safe)
K_GATHER = 1024                       # SWDGE ring-safe chunk
N_QUEUES = 4
DST_BUFS = 2                          # rotating dst gather tiles


def plan(all_src, all_dst):
    """Host layout. Returns (static_plan, per_core_data)."""
    core_of = all_src // NODES_PER_CORE
    per_core = []
    runs_len = [[None] * N_BUCKETS for _ in range(N_CORES)]
    for c in range(N_CORES):
        eidx = np.nonzero(core_of == c)[0]
        src = all_src[eidx]
        dst = all_dst[eidx]
        db = np.searchsorted(B_EDGES, dst, side="right") - 1
        order = np.lexsort((src, db))          # sort by (dst_bucket, src)
        src, dst, db, eidx = src[order], dst[order], db[order], eidx[order]
        segs = []
        for b in range(N_BUCKETS):
            m = db == b
            sb, dbv, eb = src[m], dst[m], eidx[m]
            # runs of equal src (already sorted by src within bucket)
            uniq, start, cnt = np.unique(sb, return_index=True,
                                         return_counts=True)
            # order runs by length desc (stable)
            ro = np.argsort(-cnt, kind="stable")
            segs.append((uniq[ro], start[ro], cnt[ro], sb, dbv, eb))
            runs_len[c][b] = cnt[ro]
        per_core.append(segs)

    # static caps: per bucket, group count and per-group column capacity
    ell_hat = []
    g_cap = []
    for b in range(N_BUCKETS):
        gmax = max(-(-len(runs_len[c][b]) // P) for c in range(N_CORES))
        g_cap.append(gmax)
        ell = np.zeros(gmax, dtype=np.int64)
        for c in range(N_CORES):
            cnt = runs_len[c][b]
            for g in range(-(-len(cnt) // P)):
                ell[g] = max(ell[g], cnt[g * P])   # desc-sorted: first is max
        ell_hat.append(ell)
    # dst columns per bucket (tails chunk at 128-idx granularity)
    dst_cols = [int(e.sum()) for e in ell_hat]
    src_cols = [int(g) for g in g_cap]            # one src col per group
    return (ell_hat, g_cap, dst_cols, src_cols), per_core


def _chunks(total_idx):
    out = [K_GATHER] * (total_idx // K_GATHER)
    t = total_idx % K_GATHER
    if t:
        out.append(t)                  # already multiple of 128
    return out


def build_program(static_plan, repeats=1, do_dve=True, do_gather=True):
    ell_hat, g_cap, dst_cols, src_cols = static_plan
    G_tot = sum(src_cols)
    C_tot = sum(dst_cols)
    sidx_cols = G_tot * P // 16        # wrapped idx cols
    didx_cols = C_tot * P // 16

    nc = bacc.Bacc("TRN2", target_bir_lowering=False, debug=False,
                   num_swdge_queues=N_QUEUES)
    h = nc.dram_tensor("h", [N_NODES, D], mybir.dt.bfloat16,
                       kind="ExternalInput")
    hrange = nc.dram_tensor("hrange", [RANGE_CAP, D], mybir.dt.bfloat16,
                            kind="ExternalInput")
    sidx = nc.dram_tensor("sidx", [P, sidx_cols], mybir.dt.int16,
                          kind="ExternalInput")
    didx = nc.dram_tensor("didx", [P, didx_cols], mybir.dt.int16,
                          kind="ExternalInput")
    out = nc.dram_tensor("scores", [P, C_tot], mybir.dt.float32,
                         kind="ExternalOutput")

    qn = [0]

    def q():
        qn[0] += 1
        return qn[0] % N_QUEUES

    with tile.TileContext(nc) as tc:
        with (
            tc.tile_pool(name="idxp", bufs=1) as idx_pool,
            tc.tile_pool(name="sr", bufs=1) as src_pool,
            tc.tile_pool(name="sc", bufs=1) as sc_pool,
            tc.tile_pool(name="dp", bufs=DST_BUFS) as dpool,
        ):
            sidx_t = idx_pool.tile([P, sidx_cols], mybir.dt.int16)
            didx_t = idx_pool.tile([P, didx_cols], mybir.dt.int16)
            nc.sync.dma_start(out=sidx_t[:], in_=sidx[:])
            nc.sync.dma_start(out=didx_t[:], in_=didx[:])
            scores = sc_pool.tile([P, C_tot], mybir.dt.float32)
            nc.vector.memset(scores[:], 0.0)
            src_res = src_pool.tile([P, G_tot, D], mybir.dt.bfloat16)

            max_bucket_cols = max(dst_cols)
            for _rep in range(repeats):
                # 1) src rows: one flat gather stream from hrange
                i0, col = 0, 0
                for kk in _chunks(G_tot * P):
                    ct = kk // P
                    S = kk // 16
                    if do_gather:
                        nc.gpsimd.dma_gather(
                            out_ap=src_res[:, col:col + ct, :],
                            in_ap=hrange[:, :],
                            idxs_ap=sidx_t[:, i0:i0 + S],
                            num_idxs=kk, num_idxs_reg=kk,
                            elem_size=D, queue_num=q())
                    i0 += S
                    col += ct
                if not do_gather:
                    nc.gpsimd.memset(src_res[:], 0.0)

                # 2) per dst bucket: gather dst slots, then DVE
                i0 = 0
                bcol0 = 0          # scores column base of bucket
                gcol0 = 0          # src_res column base of bucket
                for b in range(N_BUCKETS):
                    h_b = h[B_EDGES[b]:B_EDGES[b + 1], :]
                    cb = dst_cols[b]
                    dtile = dpool.tile([P, max_bucket_cols, D],
                                       mybir.dt.bfloat16, tag="d")
                    col = 0
                    for kk in _chunks(cb * P):
                        ct = kk // P
                        S = kk // 16
                        if do_gather:
                            nc.gpsimd.dma_gather(
                                out_ap=dtile[:, col:col + ct, :],
                                in_ap=h_b,
                                idxs_ap=didx_t[:, i0:i0 + S],
                                num_idxs=kk, num_idxs_reg=kk,
                                elem_size=D, queue_num=q())
                        else:
                            nc.gpsimd.memset(dtile[:, col:col + ct, :], 0.0)
                        i0 += S
                        col += ct
                    if do_dve:
                        # group spans: merge consecutive groups with l==1
                        ell = ell_hat[b]
                        g = 0
                        dcol = 0
                        while g < len(ell):
                            l = int(ell[g])
                            if l == 1:
                                m = len(ell) - g     # desc-sorted: rest are 1
                                d_sl = dtile[:, dcol:dcol + m, :]
                                s_sl = src_res[:, gcol0 + g:gcol0 + g + m, :]
                                nc.vector.tensor_mul(out=d_sl, in0=d_sl,
                                                     in1=s_sl)
                                nc.vector.tensor_reduce(
                                    out=scores[:, bcol0 + dcol:
                                               bcol0 + dcol + m],
                                    in_=d_sl, axis=mybir.AxisListType.X,
                                    op=mybir.AluOpType.add)
                                dcol += m
                                g += m
                            else:
                                d_sl = dtile[:, dcol:dcol + l, :]
                                s_sl = src_res[:, gcol0 + g:gcol0 + g + 1, :]
                                nc.vector.tensor_mul(
                                    out=d_sl, in0=d_sl,
                                    in1=s_sl.to_broadcast([P, l, D]))
                                nc.vector.tensor_reduce(
                                    out=scores[:, bcol0 + dcol:
                                               bcol0 + dcol + l],
                                    in_=d_sl, axis=mybir.AxisListType.X,
                                    op=mybir.AluOpType.add)
                                dcol += l
                                g += 1
                    bcol0 += cb
                    gcol0 += g_cap[b]

            nc.sync.dma_start(out=out[:], in_=scores[:])
    nc.compile()
    return nc


def _wrap_block(flat_i16):
    k = flat_i16.shape[0]
    w = flat_i16.reshape(k // 16, 16).T
    return np.ascontiguousarray(np.tile(w, (8, 1)))


def _wrap_stream(flat_i16):
    segs = []
    off = 0
    for kk in _chunks(flat_i16.shape[0]):
        segs.append(_wrap_block(flat_i16[off:off + kk]))
        off += kk
    return np.concatenate(segs, axis=1)


def make_core_inputs(core_id, segs, static_plan, h):
    ell_hat, g_cap, dst_cols, src_cols = static_plan
    h16 = np.asarray(h, dtype=ml_dtypes.bfloat16)
    lo = core_id * NODES_PER_CORE
    hr = np.zeros((RANGE_CAP, D), dtype=ml_dtypes.bfloat16)
    hi = min(lo + RANGE_CAP, N_NODES)
    hr[:hi - lo] = h16[lo:hi]

    G_tot = sum(g_cap)
    C_tot = sum(dst_cols)
    src_flat = (np.arange(G_tot * P, dtype=np.int64) * 97) % NODES_PER_CORE
    dst_flat = np.empty(C_tot * P, dtype=np.int64)

    e_glob = []          # original edge index per slot-entry
    e_slot_p = []
    e_slot_col = []

    gcol0 = 0
    bcol0 = 0
    for b in range(N_BUCKETS):
        uniq, start, cnt, sb, dbv, eb = segs[b]
        ell = ell_hat[b]
        nr = len(uniq)
        # src slots: run r -> (partition r%P at flat pos g*P + p)
        r = np.arange(nr)
        src_flat[gcol0 * P + r] = uniq - lo
        # dst slots, column-major within bucket
        bsz = B_EDGES[b + 1] - B_EDGES[b]
        seg = (np.arange(dst_cols[b] * P, dtype=np.int64) * 89) % bsz
        col_of_g = np.concatenate([[0], np.cumsum(ell)])
        for g in range(-(-nr // P)):
            rr = np.arange(g * P, min((g + 1) * P, nr))
            for j in range(int(ell[g])):
                sel = rr[cnt[rr] > j]
                if len(sel) == 0:
                    continue
                p = sel % P
                col = col_of_g[g] + j
                eix = start[sel] + j
                seg[col * P + p] = dbv[eix] - B_EDGES[b]
                e_glob.append(eb[eix])
                e_slot_p.append(p)
                e_slot_col.append(np.full(len(p), bcol0 + col))
        dst_flat[bcol0 * P:(bcol0 + dst_cols[b]) * P] = seg
        gcol0 += g_cap[b]
        bcol0 += dst_cols[b]

    sidx = _wrap_stream(src_flat.astype(np.int16))
    didx = _wrap_stream(dst_flat.astype(np.int16))
    mapping = (np.concatenate(e_glob), np.concatenate(e_slot_p),
               np.concatenate(e_slot_col))
    return ({"h": h16, "hrange": hr, "sidx": np.ascontiguousarray(sidx),
             "didx": np.ascontiguousarray(didx)}, mapping)


def run(edge_index, h, pad_value=0):
    from concourse.bass_utils import run_bass_kernel_spmd

    h = np.ascontiguousarray(np.asarray(h), dtype=np.float32)
    all_src = np.asarray(edge_index[0], dtype=np.int64)
    all_dst = np.asarray(edge_index[1], dtype=np.int64)
    static_plan, per_core = plan(all_src, all_dst)
    nc = build_program(static_plan)

    in_maps, mappings = [], []
    for c in range(N_CORES):
        m, mapping = make_core_inputs(c, per_core[c], static_plan, h)
        in_maps.append(m)
        mappings.append(mapping)

    res = run_bass_kernel_spmd(nc, in_maps, core_ids=list(range(N_CORES)))

    out = np.empty(N_EDGES, dtype=np.float32)
    for c in range(N_CORES):
        eg, pp, cc = mappings[c]
        scores = res.results[c]["scores"]
        out[eg] = scores[pp, cc]
    return out, res


def kernel(edge_index, h):
    out, _ = run(edge_index, h)
    return out
